# revision 19
# baseline (speedup 1.0000x reference)
"""Channel-attention (CAM) Trainium2 kernel.

Problem: out[b] = softmax(b_f[b] @ c_f[b].T, axis=-1) @ a_f[b] + a_f[b]
with a,b,c: [16, 1024, 32, 32] fp32, flattened to [16, 1024, 1024].

Sharding: pure data parallel over batch — 16 samples / 8 cores = 2 per core.

Per-core pipeline (per sample), fp16 compute:
  - b,c loaded via gpsimd cast-DMA (DRAM fp32 -> SBUF fp16, two row-tiles
    per transfer, deep staging pool so sample s+1 prefetches fully under
    sample s compute; s+1's load emission is interleaved into s's compute
    steps so the PE never drains at the sample boundary), then
    PE-transposed (fp16) into the merged [HW, C] operand tensor bcT
  - a loaded via cast-DMA directly into its natural-layout fp16 tile
  - software-pipelined i-loop (skew 1): PE order is
      m1(i+1) kk0-3 | E-transpose(i) | m1(i+1) kk4-7 | m2(i)
    so the softmax (DVE reduce + ACT Exp) and the ET psum->SBUF copy of
    step i hide entirely under m1(i+1)'s matmuls
  - m1: scores = bT.T @ cT, fp16, fp32 PSUM accumulation into a single
    two-bank [128,1024] psum tile (pairs share the stationary operand)
  - softmax: single DVE row-max over 1024, ACT Exp with bias=-max and
    accum_out row-sum; the 1/sum division is deferred to the output
  - m2: out = ET.T @ a16, fp32 PSUM accumulation, single two-bank tile
  - finalize: one DVE scalar_tensor_tensor out = psum * (1/sum) + a16
    (residual uses the fp16 a, avoiding a second fp32 load of a), store
    on the sync queue so the scalar queue never blocks the Exps

Note: PE never executes fp32 ops — fp32 transpose-mode matmuls were
observed to hang the PE intermittently when interleaved with 16-bit
FWL-eligible matmul streams.
"""
import os
import sys
import types

import numpy as np


def _install_axon_hooks():
    """Provide antenv.axon_hooks (missing in this image) so trace=True works."""
    if 'antenv.axon_hooks' in sys.modules:
        return
    m = types.ModuleType('antenv.axon_hooks')
    m._hook = None
    m.set_axon_ntff_profile_hook = lambda h: setattr(m, '_hook', h)
    m.get_axon_ntff_profile_hook = lambda: m._hook
    sys.modules['antenv.axon_hooks'] = m
    try:
        import antenv
        antenv.axon_hooks = m
    except ImportError:
        pass
    try:
        from trn_agent_boot.trn_boot import _ntff_profile_via_ctypes
        m.set_axon_ntff_profile_hook(
            _ntff_profile_via_ctypes('/opt/axon/libaxon_pjrt.so'))
    except Exception:
        pass


_install_axon_hooks()

import concourse.bass as bass  # noqa: E402
import concourse.mybir as mybir  # noqa: E402
import concourse.tile as tile  # noqa: E402
from concourse import bacc, bass_utils  # noqa: E402
from concourse.masks import make_identity  # noqa: E402

# artifact upload needs a bucket; keep everything local in the sandbox
bass_utils.upload_artifacts = lambda tmpdir: f"local:{tmpdir}"

N_CORES = 8
B, C, H, W = 16, 1024, 32, 32
HW = H * W
S = B // N_CORES        # samples per core
P = 128
NT = C // P             # 8 row tiles
F32 = mybir.dt.float32
F16 = mybir.dt.float16
ALU = mybir.AluOpType
AX = mybir.AxisListType
ACTF = mybir.ActivationFunctionType

# DMA-xbar transpose for later samples' c operand (staged fp16 in DRAM
# during the previous sample's compute) — removes 64 PE transposes/sample
XBAR = os.environ.get("CAM_XBAR", "1") == "1"


def cam_kernel(ctx, tc, out_ap, a_ap, b_ap, c_ap, n_samples=S):
    nc = tc.nc

    const_pool = ctx.enter_context(tc.tile_pool(name="const", bufs=1))
    natp = ctx.enter_context(tc.tile_pool(name="nat", bufs=8))
    bigp = ctx.enter_context(tc.tile_pool(name="big", bufs=2))
    a16p = ctx.enter_context(tc.tile_pool(name="a16", bufs=2))
    etp = ctx.enter_context(tc.tile_pool(name="et", bufs=NT + 1))
    ep = ctx.enter_context(tc.tile_pool(name="E", bufs=2))
    otp = ctx.enter_context(tc.tile_pool(name="ot", bufs=2))
    smp = ctx.enter_context(tc.tile_pool(name="sm", bufs=4))
    rip = ctx.enter_context(tc.tile_pool(name="ri", bufs=NT + 1))
    pt_pool = ctx.enter_context(tc.tile_pool(name="pt", bufs=2, space="PSUM"))
    ps_pool = ctx.enter_context(tc.tile_pool(name="ps", bufs=2, space="PSUM"))
    drp = ctx.enter_context(tc.tile_pool(name="dr", bufs=2, space="DRAM"))

    ident = const_pool.tile([P, P], F16)
    make_identity(nc, ident[:])

    # per-sample persistent tiles
    bcTs = []
    a16s = []
    for s in range(n_samples):
        bcT = bigp.tile([P, NT, 2 * C], F16, tag="bcT", name=f"bcT{s}")
        a16 = a16p.tile([P, NT, HW], F16, tag="a16", name=f"a16{s}")
        bcTs.append(bcT)
        a16s.append(a16)

    def load_pair(s, src_ap, r, base):
        """Cast-load row-tiles r,r+1 of src and transpose into bcT."""
        bcT = bcTs[s]
        nat = natp.tile([P, 2, HW], F16, tag="nat", name=f"nat{s}_{base}_{r}")
        src = src_ap[s, r * P:(r + 2) * P, :].rearrange(
            "(two p) hw -> p two hw", two=2)
        nc.gpsimd.dma_start(nat[:], src)
        for t in range(2):
            pt = pt_pool.tile([P, NT * P], F16, tag="pt", name=f"pt{s}{base}{r}{t}")
            for j in range(NT):
                nc.tensor.transpose(
                    pt[:, j * P:(j + 1) * P],
                    nat[:, t, j * P:(j + 1) * P], ident[:])
            nc.scalar.copy(
                bcT[:, :, base + (r + t) * P:base + (r + t + 1) * P],
                pt[:].rearrange("p (t c) -> p t c", t=NT))

    def load_a(s, r):
        a16 = a16s[s]
        dst = a16[:, r:r + 2, :]
        src = a_ap[s, r * P:(r + 2) * P, :].rearrange(
            "(two p) hw -> p two hw", two=2)
        nc.gpsimd.dma_start(dst, src)

    def bc_loads(s, xbar_c=False):
        """DMA(+transpose) emitters for b,c: b r0-1 + all c (phase-A
        prerequisites for m1(0..1)), then b r2..7 (one per m1 step).

        With xbar_c, c is instead cast-staged fp16 into DRAM and then
        transpose-loaded by the DMA xbar straight into bcT (no PE work).
        """
        if not xbar_c:
            yield lambda: load_pair(s, b_ap, 0, 0)
            for r in range(0, NT, 2):
                yield (lambda r=r: load_pair(s, c_ap, r, C))
            for r in range(2, NT, 2):
                yield (lambda r=r: load_pair(s, b_ap, r, 0))
            return
        bcT = bcTs[s]
        scr = drp.tile([C, HW], F16, tag="scr", name=f"scr{s}")

        def stage_c(r):
            nc.gpsimd.dma_start(scr[r * P:(r + 2) * P, :],
                                c_ap[s, r * P:(r + 2) * P, :])

        def xbar_all():
            for kk in range(NT):
                nc.sync.dma_start(bcT[:, kk, C:2 * C],
                                  scr[:, kk * P:(kk + 1) * P],
                                  transpose=True)

        yield lambda: load_pair(s, b_ap, 0, 0)
        for r in range(0, NT, 2):
            yield (lambda r=r: stage_c(r))
        yield xbar_all
        for r in range(2, NT, 2):
            yield (lambda r=r: load_pair(s, b_ap, r, 0))

    def a_loads(s):
        for r in range(0, NT, 2):
            load_a(s, r)

    def compute_steps(s, next_loads):
        """Phase A: m1 + softmax + ET for all row tiles (skew-1 pipeline);
        phase B: all m2 + finalize back-to-back. The next sample's load
        emission is interleaved into the steps so its DMAs prefetch and
        its transposes fill PE slack."""
        bcT = bcTs[s]
        a16 = a16s[s]
        ps = {}
        state = {}
        own_brest = own_loads.get(s, [])

        def emit_m1_kk(i, kk0, kk1):
            tgt = ps[i]
            for kk in range(kk0, kk1):
                lhsT = bcT[:, kk, i * P:(i + 1) * P]
                nc.tensor.matmul(tgt[:, 0:512], lhsT,
                                 bcT[:, kk, C:C + 512],
                                 start=(kk == 0), stop=(kk == NT - 1))
                nc.tensor.matmul(tgt[:, 512:1024], lhsT,
                                 bcT[:, kk, C + 512:C + 1024],
                                 start=(kk == 0), stop=(kk == NT - 1))

        for i in range(-1, NT):
            inx = i + 1
            if inx < NT:
                ps[inx] = ps_pool.tile([P, 1024], F32, tag="ps",
                                       name=f"ps{s}_{inx}")
                emit_m1_kk(inx, 0, 6)
            if i >= 0:
                # softmax(i) on DVE+ACT (runs while PE does m1(i+1))
                pst = ps.pop(i)
                nmx = smp.tile([P, 1], F32, tag="sc", name=f"nmx{s}_{i}")
                nc.vector.tensor_reduce(nmx[:], pst[:], axis=AX.X, op=ALU.max)
                nc.vector.tensor_scalar_mul(nmx[:], nmx[:], -1.0)

                E = ep.tile([P, C], F16, tag="E", name=f"E{s}_{i}")
                rinv = rip.tile([P, 1], F32, tag="ri", name=f"ri{s}_{i}")
                nc.scalar.activation(E[:], pst[:], ACTF.Exp,
                                     bias=nmx[:], scale=1.0, accum_out=rinv[:])

                # E^T on PE (between the two m1(i+1) chunks)
                pt = pt_pool.tile([P, NT * P], F16, tag="pt", name=f"ptE{s}_{i}")
                for j in range(NT):
                    nc.tensor.transpose(
                        pt[:, j * P:(j + 1) * P],
                        E[:, j * P:(j + 1) * P], ident[:])
                ET = etp.tile([P, NT, P], F16, tag="ET", name=f"ET{s}_{i}")
                nc.scalar.copy(
                    ET[:], pt[:].rearrange("p (t c) -> p t c", t=NT))
                nc.vector.reciprocal(rinv[:], rinv[:])
                state[i] = (ET, rinv)
            if inx < NT:
                emit_m1_kk(inx, 6, NT)
            # own trailing b tiles: transposes paced one per early step,
            # just behind their DMA arrivals (sample 0 only)
            if own_brest:
                own_brest.pop(0)()
                if not own_brest:
                    a_loads(s)   # a: pure DMA issues, gate only phase B

        # ---- phase B: m2 burst + finalize ----
        for i in range(NT):
            ET, rinv = state.pop(i)
            po = ps_pool.tile([P, 1024], F32, tag="ps", name=f"po{s}_{i}")
            for jj in range(NT):
                first, last = jj == 0, jj == NT - 1
                l_e = ET[:, jj, :]
                nc.tensor.matmul(po[:, 0:512], l_e, a16[:, jj, 0:512],
                                 start=first, stop=last)
                nc.tensor.matmul(po[:, 512:1024], l_e, a16[:, jj, 512:1024],
                                 start=first, stop=last)

            isl = slice(i * P, (i + 1) * P)
            ot = otp.tile([P, HW], F32, tag="ot", name=f"ot{s}_{i}")
            if s == n_samples - 1 and i == NT - 1:
                # last tile: split finalize+store in halves to shorten the tail
                for h in range(2):
                    hsl = slice(h * 512, (h + 1) * 512)
                    nc.vector.scalar_tensor_tensor(
                        ot[:, hsl], po[:, hsl], rinv[:], a16[:, i, hsl],
                        op0=ALU.mult, op1=ALU.add)
                    nc.sync.dma_start(out_ap[s, isl, hsl], ot[:, hsl])
            else:
                nc.vector.scalar_tensor_tensor(
                    ot[:], po[:], rinv[:], a16[:, i, :],
                    op0=ALU.mult, op1=ALU.add)
                nc.sync.dma_start(out_ap[s, isl, :], ot[:])
            # next sample's b/c groups: one per m2 step (8 groups, 8 steps)
            nl = next(next_loads, None)
            if nl is not None:
                nl()

    # sample 0: b r0-1 + c upfront (nothing to overlap with); its b r2..7
    # are paced into its own early phase-A steps
    own_loads = {}
    l0 = list(bc_loads(0))
    for emit in l0[:5]:
        emit()
    own_loads[0] = l0[5:]
    for s in range(n_samples):
        if s + 1 < n_samples:
            nxt = iter(list(bc_loads(s + 1, xbar_c=XBAR)))
        else:
            nxt = iter(())
        compute_steps(s, nxt)
        if s + 1 < n_samples:
            for nl in nxt:   # any b/c groups not yet emitted
                nl()
            a_loads(s + 1)


_BUILT = {}


def build_program(n_samples=S):
    key = n_samples
    if key in _BUILT:
        return _BUILT[key]
    nc = bacc.Bacc("TRN2", target_bir_lowering=False, debug=False,
                   enable_asserts=False, num_devices=N_CORES)
    a = nc.dram_tensor("a", [S, C, HW], F32, kind="ExternalInput").ap()
    b = nc.dram_tensor("b", [S, C, HW], F32, kind="ExternalInput").ap()
    c = nc.dram_tensor("c", [S, C, HW], F32, kind="ExternalInput").ap()
    out = nc.dram_tensor("out", [S, C, HW], F32, kind="ExternalOutput").ap()
    from contextlib import ExitStack
    with tile.TileContext(nc) as tc, ExitStack() as ctx:
        cam_kernel(ctx, tc, out, a, b, c, n_samples=n_samples)
    nc.compile()
    _BUILT[key] = nc
    return nc


def run_sharded(a, b, c, trace=False, n_samples=S, **kw):
    """a,b,c: [16,1024,1024] fp32 -> (full output, BassKernelResults)."""
    nc = build_program(n_samples)
    in_maps = []
    for core in range(N_CORES):
        sl = slice(core * S, (core + 1) * S)
        in_maps.append({"a": np.ascontiguousarray(a[sl]),
                        "b": np.ascontiguousarray(b[sl]),
                        "c": np.ascontiguousarray(c[sl])})
    res = bass_utils.run_bass_kernel_spmd(
        nc, in_maps, core_ids=list(range(N_CORES)), trace=trace, **kw)
    out = np.concatenate([res.results[core]["out"] for core in range(N_CORES)],
                         axis=0)
    return out, res


def kernel(a, b, c):
    a = np.asarray(a, dtype=np.float32).reshape(B, C, HW)
    b = np.asarray(b, dtype=np.float32).reshape(B, C, HW)
    c = np.asarray(c, dtype=np.float32).reshape(B, C, HW)
    out, _ = run_sharded(a, b, c, trace=False)
    return out.reshape(B, C, H, W)


# revision 21
# speedup vs baseline: 1.2220x; 1.2220x over previous
"""Channel-attention (CAM) Trainium2 kernel.

Problem: out[b] = softmax(b_f[b] @ c_f[b].T, axis=-1) @ a_f[b] + a_f[b]
with a,b,c: [16, 1024, 32, 32] fp32, flattened to [16, 1024, 1024].

Sharding: pure data parallel over batch — 16 samples / 8 cores = 2 per core.

Per-core pipeline (per sample), fp16 compute:
  - b,c loaded via gpsimd cast-DMA (DRAM fp32 -> SBUF fp16, two row-tiles
    per transfer, deep staging pool so sample s+1 prefetches fully under
    sample s compute; s+1's load emission is interleaved into s's compute
    steps so the PE never drains at the sample boundary), then
    PE-transposed (fp16) into the merged [HW, C] operand tensor bcT
  - a loaded via cast-DMA directly into its natural-layout fp16 tile
  - software-pipelined i-loop (skew 1): PE order is
      m1(i+1) kk0-3 | E-transpose(i) | m1(i+1) kk4-7 | m2(i)
    so the softmax (DVE reduce + ACT Exp) and the ET psum->SBUF copy of
    step i hide entirely under m1(i+1)'s matmuls
  - m1: scores = bT.T @ cT, fp16, fp32 PSUM accumulation into a single
    two-bank [128,1024] psum tile (pairs share the stationary operand)
  - softmax: single DVE row-max over 1024, ACT Exp with bias=-max and
    accum_out row-sum; the 1/sum division is deferred to the output
  - m2: out = ET.T @ a16, fp32 PSUM accumulation, single two-bank tile
  - finalize: one DVE scalar_tensor_tensor out = psum * (1/sum) + a16
    (residual uses the fp16 a, avoiding a second fp32 load of a), store
    on the sync queue so the scalar queue never blocks the Exps

Note: PE never executes fp32 ops — fp32 transpose-mode matmuls were
observed to hang the PE intermittently when interleaved with 16-bit
FWL-eligible matmul streams.
"""
import os
import sys
import types

import numpy as np


def _install_axon_hooks():
    """Provide antenv.axon_hooks (missing in this image) so trace=True works."""
    if 'antenv.axon_hooks' in sys.modules:
        return
    m = types.ModuleType('antenv.axon_hooks')
    m._hook = None
    m.set_axon_ntff_profile_hook = lambda h: setattr(m, '_hook', h)
    m.get_axon_ntff_profile_hook = lambda: m._hook
    sys.modules['antenv.axon_hooks'] = m
    try:
        import antenv
        antenv.axon_hooks = m
    except ImportError:
        pass
    try:
        from trn_agent_boot.trn_boot import _ntff_profile_via_ctypes
        m.set_axon_ntff_profile_hook(
            _ntff_profile_via_ctypes('/opt/axon/libaxon_pjrt.so'))
    except Exception:
        pass


_install_axon_hooks()

import concourse.bass as bass  # noqa: E402
import concourse.mybir as mybir  # noqa: E402
import concourse.tile as tile  # noqa: E402
from concourse import bacc, bass_utils  # noqa: E402
from concourse.masks import make_identity  # noqa: E402

# artifact upload needs a bucket; keep everything local in the sandbox
bass_utils.upload_artifacts = lambda tmpdir: f"local:{tmpdir}"

N_CORES = 8
B, C, H, W = 16, 1024, 32, 32
HW = H * W
S = B // N_CORES        # samples per core
P = 128
NT = C // P             # 8 row tiles
F32 = mybir.dt.float32
F16 = mybir.dt.float16
ALU = mybir.AluOpType
AX = mybir.AxisListType
ACTF = mybir.ActivationFunctionType

# DMA-xbar transpose for later samples' c operand (staged fp16 in DRAM
# during the previous sample's compute) — removes 64 PE transposes/sample
XBAR = os.environ.get("CAM_XBAR", "0") == "1"


def cam_kernel(ctx, tc, out_ap, a_ap, b_ap, c_ap, n_samples=S):
    nc = tc.nc

    const_pool = ctx.enter_context(tc.tile_pool(name="const", bufs=1))
    natp = ctx.enter_context(tc.tile_pool(name="nat", bufs=8))
    bigp = ctx.enter_context(tc.tile_pool(name="big", bufs=2))
    a16p = ctx.enter_context(tc.tile_pool(name="a16", bufs=2))
    etp = ctx.enter_context(tc.tile_pool(name="et", bufs=NT + 1))
    ep = ctx.enter_context(tc.tile_pool(name="E", bufs=2))
    otp = ctx.enter_context(tc.tile_pool(name="ot", bufs=2))
    smp = ctx.enter_context(tc.tile_pool(name="sm", bufs=4))
    rip = ctx.enter_context(tc.tile_pool(name="ri", bufs=NT + 1))
    pt_pool = ctx.enter_context(tc.tile_pool(name="pt", bufs=2, space="PSUM"))
    ps_pool = ctx.enter_context(tc.tile_pool(name="ps", bufs=2, space="PSUM"))
    drp = ctx.enter_context(tc.tile_pool(name="dr", bufs=2, space="DRAM"))

    ident = const_pool.tile([P, P], F16)
    make_identity(nc, ident[:])

    # per-sample persistent tiles
    bcTs = []
    a16s = []
    for s in range(n_samples):
        bcT = bigp.tile([P, NT, 2 * C], F16, tag="bcT", name=f"bcT{s}")
        a16 = a16p.tile([P, NT, HW], F16, tag="a16", name=f"a16{s}")
        bcTs.append(bcT)
        a16s.append(a16)

    def load_pair(s, src_ap, r, base):
        """Cast-load row-tiles r,r+1 of src and transpose into bcT."""
        bcT = bcTs[s]
        nat = natp.tile([P, 2, HW], F16, tag="nat", name=f"nat{s}_{base}_{r}")
        src = src_ap[s, r * P:(r + 2) * P, :].rearrange(
            "(two p) hw -> p two hw", two=2)
        nc.gpsimd.dma_start(nat[:], src)
        for t in range(2):
            pt = pt_pool.tile([P, NT * P], F16, tag="pt", name=f"pt{s}{base}{r}{t}")
            for j in range(NT):
                nc.tensor.transpose(
                    pt[:, j * P:(j + 1) * P],
                    nat[:, t, j * P:(j + 1) * P], ident[:])
            nc.scalar.copy(
                bcT[:, :, base + (r + t) * P:base + (r + t + 1) * P],
                pt[:].rearrange("p (t c) -> p t c", t=NT))

    def load_a(s, r):
        a16 = a16s[s]
        dst = a16[:, r:r + 2, :]
        src = a_ap[s, r * P:(r + 2) * P, :].rearrange(
            "(two p) hw -> p two hw", two=2)
        nc.gpsimd.dma_start(dst, src)

    def bc_loads(s, xbar_c=False):
        """DMA(+transpose) emitters for b,c: b r0-1 + all c (phase-A
        prerequisites for m1(0..1)), then b r2..7 (one per m1 step).

        With xbar_c, c is instead cast-staged fp16 into DRAM and then
        transpose-loaded by the DMA xbar straight into bcT (no PE work).
        """
        if not xbar_c:
            yield lambda: load_pair(s, b_ap, 0, 0)
            for r in range(0, NT, 2):
                yield (lambda r=r: load_pair(s, c_ap, r, C))
            for r in range(2, NT, 2):
                yield (lambda r=r: load_pair(s, b_ap, r, 0))
            return
        bcT = bcTs[s]
        scr = drp.tile([C, HW], F16, tag="scr", name=f"scr{s}")

        def stage_c(r):
            nc.gpsimd.dma_start(scr[r * P:(r + 2) * P, :],
                                c_ap[s, r * P:(r + 2) * P, :])

        def xbar_all():
            for kk in range(NT):
                nc.sync.dma_start(bcT[:, kk, C:2 * C],
                                  scr[:, kk * P:(kk + 1) * P],
                                  transpose=True)

        yield lambda: load_pair(s, b_ap, 0, 0)
        for r in range(0, NT, 2):
            yield (lambda r=r: stage_c(r))
        yield xbar_all
        for r in range(2, NT, 2):
            yield (lambda r=r: load_pair(s, b_ap, r, 0))

    def a_loads(s):
        for r in range(0, NT, 2):
            load_a(s, r)

    def compute_steps(s, next_loads):
        """Phase A: m1 + softmax + ET for all row tiles (skew-1 pipeline);
        phase B: all m2 + finalize back-to-back. The next sample's load
        emission is interleaved into the steps so its DMAs prefetch and
        its transposes fill PE slack."""
        bcT = bcTs[s]
        a16 = a16s[s]
        ps = {}
        state = {}
        own_brest = own_loads.get(s, [])

        def emit_m1_kk(i, kk0, kk1):
            tgt = ps[i]
            for kk in range(kk0, kk1):
                lhsT = bcT[:, kk, i * P:(i + 1) * P]
                nc.tensor.matmul(tgt[:, 0:512], lhsT,
                                 bcT[:, kk, C:C + 512],
                                 start=(kk == 0), stop=(kk == NT - 1))
                nc.tensor.matmul(tgt[:, 512:1024], lhsT,
                                 bcT[:, kk, C + 512:C + 1024],
                                 start=(kk == 0), stop=(kk == NT - 1))

        for i in range(-1, NT):
            inx = i + 1
            if inx < NT:
                ps[inx] = ps_pool.tile([P, 1024], F32, tag="ps",
                                       name=f"ps{s}_{inx}")
                emit_m1_kk(inx, 0, 6)
            if i >= 0:
                # softmax(i) on DVE+ACT (runs while PE does m1(i+1))
                pst = ps.pop(i)
                nmx = smp.tile([P, 1], F32, tag="sc", name=f"nmx{s}_{i}")
                nc.vector.tensor_reduce(nmx[:], pst[:], axis=AX.X, op=ALU.max)
                nc.vector.tensor_scalar_mul(nmx[:], nmx[:], -1.0)

                E = ep.tile([P, C], F16, tag="E", name=f"E{s}_{i}")
                rinv = rip.tile([P, 1], F32, tag="ri", name=f"ri{s}_{i}")
                nc.scalar.activation(E[:], pst[:], ACTF.Exp,
                                     bias=nmx[:], scale=1.0, accum_out=rinv[:])

                # E^T on PE (between the two m1(i+1) chunks)
                pt = pt_pool.tile([P, NT * P], F16, tag="pt", name=f"ptE{s}_{i}")
                for j in range(NT):
                    nc.tensor.transpose(
                        pt[:, j * P:(j + 1) * P],
                        E[:, j * P:(j + 1) * P], ident[:])
                ET = etp.tile([P, NT, P], F16, tag="ET", name=f"ET{s}_{i}")
                nc.scalar.copy(
                    ET[:], pt[:].rearrange("p (t c) -> p t c", t=NT))
                nc.vector.reciprocal(rinv[:], rinv[:])
                state[i] = (ET, rinv)
            if inx < NT:
                emit_m1_kk(inx, 6, NT)
            # own trailing b tiles: transposes paced one per early step,
            # just behind their DMA arrivals (sample 0 only)
            if own_brest:
                own_brest.pop(0)()
                if not own_brest:
                    a_loads(s)   # a: pure DMA issues, gate only phase B

        # ---- phase B: m2 burst + finalize ----
        for i in range(NT):
            ET, rinv = state.pop(i)
            po = ps_pool.tile([P, 1024], F32, tag="ps", name=f"po{s}_{i}")
            for jj in range(NT):
                first, last = jj == 0, jj == NT - 1
                l_e = ET[:, jj, :]
                nc.tensor.matmul(po[:, 0:512], l_e, a16[:, jj, 0:512],
                                 start=first, stop=last)
                nc.tensor.matmul(po[:, 512:1024], l_e, a16[:, jj, 512:1024],
                                 start=first, stop=last)

            isl = slice(i * P, (i + 1) * P)
            ot = otp.tile([P, HW], F32, tag="ot", name=f"ot{s}_{i}")
            if s == n_samples - 1 and i >= NT - 2:
                # last tile: split finalize+store in halves to shorten the tail
                for h in range(2):
                    hsl = slice(h * 512, (h + 1) * 512)
                    nc.vector.scalar_tensor_tensor(
                        ot[:, hsl], po[:, hsl], rinv[:], a16[:, i, hsl],
                        op0=ALU.mult, op1=ALU.add)
                    nc.sync.dma_start(out_ap[s, isl, hsl], ot[:, hsl])
            else:
                nc.vector.scalar_tensor_tensor(
                    ot[:], po[:], rinv[:], a16[:, i, :],
                    op0=ALU.mult, op1=ALU.add)
                nc.sync.dma_start(out_ap[s, isl, :], ot[:])
            # next sample's b/c groups: one per m2 step (8 groups, 8 steps)
            nl = next(next_loads, None)
            if nl is not None:
                nl()

    # sample 0: b r0-1 + c upfront (nothing to overlap with); its b r2..7
    # are paced into its own early phase-A steps
    own_loads = {}
    l0 = list(bc_loads(0))
    for emit in l0[:5]:
        emit()
    own_loads[0] = l0[5:]
    for s in range(n_samples):
        if s + 1 < n_samples:
            nxt = iter(list(bc_loads(s + 1, xbar_c=XBAR)))
        else:
            nxt = iter(())
        compute_steps(s, nxt)
        if s + 1 < n_samples:
            for nl in nxt:   # any b/c groups not yet emitted
                nl()
            a_loads(s + 1)


_BUILT = {}


def build_program(n_samples=S):
    key = n_samples
    if key in _BUILT:
        return _BUILT[key]
    nc = bacc.Bacc("TRN2", target_bir_lowering=False, debug=False,
                   enable_asserts=False, num_devices=N_CORES)
    a = nc.dram_tensor("a", [S, C, HW], F32, kind="ExternalInput").ap()
    b = nc.dram_tensor("b", [S, C, HW], F32, kind="ExternalInput").ap()
    c = nc.dram_tensor("c", [S, C, HW], F32, kind="ExternalInput").ap()
    out = nc.dram_tensor("out", [S, C, HW], F32, kind="ExternalOutput").ap()
    from contextlib import ExitStack
    with tile.TileContext(nc) as tc, ExitStack() as ctx:
        cam_kernel(ctx, tc, out, a, b, c, n_samples=n_samples)
    nc.compile()
    _BUILT[key] = nc
    return nc


def run_sharded(a, b, c, trace=False, n_samples=S, **kw):
    """a,b,c: [16,1024,1024] fp32 -> (full output, BassKernelResults)."""
    nc = build_program(n_samples)
    in_maps = []
    for core in range(N_CORES):
        sl = slice(core * S, (core + 1) * S)
        in_maps.append({"a": np.ascontiguousarray(a[sl]),
                        "b": np.ascontiguousarray(b[sl]),
                        "c": np.ascontiguousarray(c[sl])})
    res = bass_utils.run_bass_kernel_spmd(
        nc, in_maps, core_ids=list(range(N_CORES)), trace=trace, **kw)
    out = np.concatenate([res.results[core]["out"] for core in range(N_CORES)],
                         axis=0)
    return out, res


def kernel(a, b, c):
    a = np.asarray(a, dtype=np.float32).reshape(B, C, HW)
    b = np.asarray(b, dtype=np.float32).reshape(B, C, HW)
    c = np.asarray(c, dtype=np.float32).reshape(B, C, HW)
    out, _ = run_sharded(a, b, c, trace=False)
    return out.reshape(B, C, H, W)


# revision 22
# speedup vs baseline: 1.2766x; 1.0446x over previous
"""Channel-attention (CAM) Trainium2 kernel.

Problem: out[b] = softmax(b_f[b] @ c_f[b].T, axis=-1) @ a_f[b] + a_f[b]
with a,b,c: [16, 1024, 32, 32] fp32, flattened to [16, 1024, 1024].

Sharding: pure data parallel over batch — 16 samples / 8 cores = 2 per core.

Per-core pipeline (per sample), fp16 compute:
  - b,c loaded via gpsimd cast-DMA (DRAM fp32 -> SBUF fp16, two row-tiles
    per transfer, deep staging pool so sample s+1 prefetches fully under
    sample s compute; s+1's load emission is interleaved into s's compute
    steps so the PE never drains at the sample boundary), then
    PE-transposed (fp16) into the merged [HW, C] operand tensor bcT
  - a loaded via cast-DMA directly into its natural-layout fp16 tile
  - software-pipelined i-loop (skew 1): PE order is
      m1(i+1) kk0-3 | E-transpose(i) | m1(i+1) kk4-7 | m2(i)
    so the softmax (DVE reduce + ACT Exp) and the ET psum->SBUF copy of
    step i hide entirely under m1(i+1)'s matmuls
  - m1: scores = bT.T @ cT, fp16, fp32 PSUM accumulation into a single
    two-bank [128,1024] psum tile (pairs share the stationary operand)
  - softmax: single DVE row-max over 1024, ACT Exp with bias=-max and
    accum_out row-sum; the 1/sum division is deferred to the output
  - m2: out = ET.T @ a16, fp32 PSUM accumulation, single two-bank tile
  - finalize: one DVE scalar_tensor_tensor out = psum * (1/sum) + a16
    (residual uses the fp16 a, avoiding a second fp32 load of a), store
    on the sync queue so the scalar queue never blocks the Exps

Note: PE never executes fp32 ops — fp32 transpose-mode matmuls were
observed to hang the PE intermittently when interleaved with 16-bit
FWL-eligible matmul streams.
"""
import os
import sys
import types

import numpy as np


def _install_axon_hooks():
    """Provide antenv.axon_hooks (missing in this image) so trace=True works."""
    if 'antenv.axon_hooks' in sys.modules:
        return
    m = types.ModuleType('antenv.axon_hooks')
    m._hook = None
    m.set_axon_ntff_profile_hook = lambda h: setattr(m, '_hook', h)
    m.get_axon_ntff_profile_hook = lambda: m._hook
    sys.modules['antenv.axon_hooks'] = m
    try:
        import antenv
        antenv.axon_hooks = m
    except ImportError:
        pass
    try:
        from trn_agent_boot.trn_boot import _ntff_profile_via_ctypes
        m.set_axon_ntff_profile_hook(
            _ntff_profile_via_ctypes('/opt/axon/libaxon_pjrt.so'))
    except Exception:
        pass


_install_axon_hooks()

import concourse.bass as bass  # noqa: E402
import concourse.mybir as mybir  # noqa: E402
import concourse.tile as tile  # noqa: E402
from concourse import bacc, bass_utils  # noqa: E402
from concourse.masks import make_identity  # noqa: E402

# artifact upload needs a bucket; keep everything local in the sandbox
bass_utils.upload_artifacts = lambda tmpdir: f"local:{tmpdir}"

N_CORES = 8
B, C, H, W = 16, 1024, 32, 32
HW = H * W
S = B // N_CORES        # samples per core
P = 128
NT = C // P             # 8 row tiles
F32 = mybir.dt.float32
F16 = mybir.dt.float16
ALU = mybir.AluOpType
AX = mybir.AxisListType
ACTF = mybir.ActivationFunctionType

# DMA-xbar transpose for later samples' c operand (staged fp16 in DRAM
# during the previous sample's compute) — removes 64 PE transposes/sample
XBAR = os.environ.get("CAM_XBAR", "0") == "1"


def cam_kernel(ctx, tc, out_ap, a_ap, b_ap, c_ap, n_samples=S):
    nc = tc.nc

    const_pool = ctx.enter_context(tc.tile_pool(name="const", bufs=1))
    natp = ctx.enter_context(tc.tile_pool(name="nat", bufs=8))
    bigp = ctx.enter_context(tc.tile_pool(name="big", bufs=2))
    a16p = ctx.enter_context(tc.tile_pool(name="a16", bufs=2))
    etp = ctx.enter_context(tc.tile_pool(name="et", bufs=NT + 1))
    ep = ctx.enter_context(tc.tile_pool(name="E", bufs=2))
    otp = ctx.enter_context(tc.tile_pool(name="ot", bufs=2))
    smp = ctx.enter_context(tc.tile_pool(name="sm", bufs=4))
    rip = ctx.enter_context(tc.tile_pool(name="ri", bufs=NT + 1))
    pt_pool = ctx.enter_context(tc.tile_pool(name="pt", bufs=2, space="PSUM"))
    ps_pool = ctx.enter_context(tc.tile_pool(name="ps", bufs=2, space="PSUM"))
    drp = ctx.enter_context(tc.tile_pool(name="dr", bufs=2, space="DRAM"))

    ident = const_pool.tile([P, P], F16)
    make_identity(nc, ident[:])

    # per-sample persistent tiles
    bcTs = []
    a16s = []
    for s in range(n_samples):
        bcT = bigp.tile([P, NT, 2 * C], F16, tag="bcT", name=f"bcT{s}")
        a16 = a16p.tile([P, NT, HW], F16, tag="a16", name=f"a16{s}")
        bcTs.append(bcT)
        a16s.append(a16)

    def load_pair(s, src_ap, r, base):
        """Cast-load row-tiles r,r+1 of src and transpose into bcT."""
        bcT = bcTs[s]
        nat = natp.tile([P, 2, HW], F16, tag="nat", name=f"nat{s}_{base}_{r}")
        src = src_ap[s, r * P:(r + 2) * P, :].rearrange(
            "(two p) hw -> p two hw", two=2)
        nc.gpsimd.dma_start(nat[:], src)
        for t in range(2):
            pt = pt_pool.tile([P, NT * P], F16, tag="pt", name=f"pt{s}{base}{r}{t}")
            for j in range(NT):
                nc.tensor.transpose(
                    pt[:, j * P:(j + 1) * P],
                    nat[:, t, j * P:(j + 1) * P], ident[:])
            nc.scalar.copy(
                bcT[:, :, base + (r + t) * P:base + (r + t + 1) * P],
                pt[:].rearrange("p (t c) -> p t c", t=NT))

    def load_a(s, r):
        a16 = a16s[s]
        dst = a16[:, r:r + 2, :]
        src = a_ap[s, r * P:(r + 2) * P, :].rearrange(
            "(two p) hw -> p two hw", two=2)
        nc.gpsimd.dma_start(dst, src)

    def bc_loads(s, xbar_c=False):
        """DMA(+transpose) emitters for b,c: b r0-1 + all c (phase-A
        prerequisites for m1(0..1)), then b r2..7 (one per m1 step).

        With xbar_c, c is instead cast-staged fp16 into DRAM and then
        transpose-loaded by the DMA xbar straight into bcT (no PE work).
        """
        if not xbar_c:
            yield lambda: load_pair(s, b_ap, 0, 0)
            for r in range(0, NT, 2):
                yield (lambda r=r: load_pair(s, c_ap, r, C))
            for r in range(2, NT, 2):
                yield (lambda r=r: load_pair(s, b_ap, r, 0))
            return
        bcT = bcTs[s]
        scr = drp.tile([C, HW], F16, tag="scr", name=f"scr{s}")

        def stage_c(r):
            nc.gpsimd.dma_start(scr[r * P:(r + 2) * P, :],
                                c_ap[s, r * P:(r + 2) * P, :])

        def xbar_all():
            for kk in range(NT):
                nc.sync.dma_start(bcT[:, kk, C:2 * C],
                                  scr[:, kk * P:(kk + 1) * P],
                                  transpose=True)

        yield lambda: load_pair(s, b_ap, 0, 0)
        for r in range(0, NT, 2):
            yield (lambda r=r: stage_c(r))
        yield xbar_all
        for r in range(2, NT, 2):
            yield (lambda r=r: load_pair(s, b_ap, r, 0))

    def a_loads(s):
        for r in range(0, NT, 2):
            load_a(s, r)

    def compute_steps(s, next_loads):
        """Phase A: m1 + softmax + ET for all row tiles (skew-1 pipeline);
        phase B: all m2 + finalize back-to-back. The next sample's load
        emission is interleaved into the steps so its DMAs prefetch and
        its transposes fill PE slack."""
        bcT = bcTs[s]
        a16 = a16s[s]
        ps = {}
        state = {}
        own_brest = own_loads.get(s, [])

        def emit_m1_kk(i, kk0, kk1):
            tgt = ps[i]
            for kk in range(kk0, kk1):
                lhsT = bcT[:, kk, i * P:(i + 1) * P]
                nc.tensor.matmul(tgt[:, 0:512], lhsT,
                                 bcT[:, kk, C:C + 512],
                                 start=(kk == 0), stop=(kk == NT - 1))
                nc.tensor.matmul(tgt[:, 512:1024], lhsT,
                                 bcT[:, kk, C + 512:C + 1024],
                                 start=(kk == 0), stop=(kk == NT - 1))

        for i in range(-1, NT):
            inx = i + 1
            if inx < NT:
                ps[inx] = ps_pool.tile([P, 1024], F32, tag="ps",
                                       name=f"ps{s}_{inx}")
                emit_m1_kk(inx, 0, 6)
            if i >= 0:
                # softmax(i) on DVE+ACT (runs while PE does m1(i+1))
                pst = ps.pop(i)
                nmx = smp.tile([P, 1], F32, tag="sc", name=f"nmx{s}_{i}")
                nc.vector.tensor_reduce(nmx[:], pst[:], axis=AX.X, op=ALU.max)
                nc.vector.tensor_scalar_mul(nmx[:], nmx[:], -1.0)

                E = ep.tile([P, C], F16, tag="E", name=f"E{s}_{i}")
                rinv = rip.tile([P, 1], F32, tag="ri", name=f"ri{s}_{i}")
                nc.scalar.activation(E[:], pst[:], ACTF.Exp,
                                     bias=nmx[:], scale=1.0, accum_out=rinv[:])

                # E^T on PE (between the two m1(i+1) chunks)
                pt = pt_pool.tile([P, NT * P], F16, tag="pt", name=f"ptE{s}_{i}")
                for j in range(NT):
                    nc.tensor.transpose(
                        pt[:, j * P:(j + 1) * P],
                        E[:, j * P:(j + 1) * P], ident[:])
                ET = etp.tile([P, NT, P], F16, tag="ET", name=f"ET{s}_{i}")
                nc.scalar.copy(
                    ET[:], pt[:].rearrange("p (t c) -> p t c", t=NT))
                nc.vector.reciprocal(rinv[:], rinv[:])
                state[i] = (ET, rinv)
            if inx < NT:
                emit_m1_kk(inx, 6, NT)
            # own trailing b tiles: transposes paced one per early step,
            # just behind their DMA arrivals (sample 0 only)
            if own_brest:
                own_brest.pop(0)()
                if not own_brest:
                    a_loads(s)   # a: pure DMA issues, gate only phase B

        # ---- phase B: m2 burst + finalize ----
        for i in range(NT):
            ET, rinv = state.pop(i)
            po = ps_pool.tile([P, 1024], F32, tag="ps", name=f"po{s}_{i}")
            for jj in range(NT):
                first, last = jj == 0, jj == NT - 1
                l_e = ET[:, jj, :]
                nc.tensor.matmul(po[:, 0:512], l_e, a16[:, jj, 0:512],
                                 start=first, stop=last)
                nc.tensor.matmul(po[:, 512:1024], l_e, a16[:, jj, 512:1024],
                                 start=first, stop=last)

            isl = slice(i * P, (i + 1) * P)
            ot = otp.tile([P, HW], F32, tag="ot", name=f"ot{s}_{i}")
            if s == n_samples - 1 and i == NT - 1:
                # last tile: split finalize+store in halves to shorten the tail
                for h in range(2):
                    hsl = slice(h * 512, (h + 1) * 512)
                    nc.vector.scalar_tensor_tensor(
                        ot[:, hsl], po[:, hsl], rinv[:], a16[:, i, hsl],
                        op0=ALU.mult, op1=ALU.add)
                    nc.sync.dma_start(out_ap[s, isl, hsl], ot[:, hsl])
            else:
                nc.vector.scalar_tensor_tensor(
                    ot[:], po[:], rinv[:], a16[:, i, :],
                    op0=ALU.mult, op1=ALU.add)
                nc.sync.dma_start(out_ap[s, isl, :], ot[:])
            # next sample's b/c groups: one per m2 step (8 groups, 8 steps)
            nl = next(next_loads, None)
            if nl is not None:
                nl()

    # sample 0: b r0-1 + c upfront (nothing to overlap with); its b r2..7
    # are paced into its own early phase-A steps
    own_loads = {}
    l0 = list(bc_loads(0))
    for emit in l0[:5]:
        emit()
    own_loads[0] = l0[5:]
    for s in range(n_samples):
        if s + 1 < n_samples:
            nxt = iter(list(bc_loads(s + 1, xbar_c=XBAR)))
        else:
            nxt = iter(())
        compute_steps(s, nxt)
        if s + 1 < n_samples:
            for nl in nxt:   # any b/c groups not yet emitted
                nl()
            a_loads(s + 1)


_BUILT = {}


def build_program(n_samples=S):
    key = n_samples
    if key in _BUILT:
        return _BUILT[key]
    nc = bacc.Bacc("TRN2", target_bir_lowering=False, debug=False,
                   enable_asserts=False, num_devices=N_CORES)
    a = nc.dram_tensor("a", [S, C, HW], F32, kind="ExternalInput").ap()
    b = nc.dram_tensor("b", [S, C, HW], F32, kind="ExternalInput").ap()
    c = nc.dram_tensor("c", [S, C, HW], F32, kind="ExternalInput").ap()
    out = nc.dram_tensor("out", [S, C, HW], F32, kind="ExternalOutput").ap()
    from contextlib import ExitStack
    with tile.TileContext(nc) as tc, ExitStack() as ctx:
        cam_kernel(ctx, tc, out, a, b, c, n_samples=n_samples)
    nc.compile()
    _BUILT[key] = nc
    return nc


def run_sharded(a, b, c, trace=False, n_samples=S, **kw):
    """a,b,c: [16,1024,1024] fp32 -> (full output, BassKernelResults)."""
    nc = build_program(n_samples)
    in_maps = []
    for core in range(N_CORES):
        sl = slice(core * S, (core + 1) * S)
        in_maps.append({"a": np.ascontiguousarray(a[sl]),
                        "b": np.ascontiguousarray(b[sl]),
                        "c": np.ascontiguousarray(c[sl])})
    res = bass_utils.run_bass_kernel_spmd(
        nc, in_maps, core_ids=list(range(N_CORES)), trace=trace, **kw)
    out = np.concatenate([res.results[core]["out"] for core in range(N_CORES)],
                         axis=0)
    return out, res


def kernel(a, b, c):
    a = np.asarray(a, dtype=np.float32).reshape(B, C, HW)
    b = np.asarray(b, dtype=np.float32).reshape(B, C, HW)
    c = np.asarray(c, dtype=np.float32).reshape(B, C, HW)
    out, _ = run_sharded(a, b, c, trace=False)
    return out.reshape(B, C, H, W)
